# revision 13
# baseline (speedup 1.0000x reference)
"""Trainium2 Bass kernel for nn_Coordinator (ragged bidirectional GRU coordinator).

Sharding: agent-parallel — 8 agents over 8 NeuronCores (one agent per core),
full batch per core. Host does the ragged pack bookkeeping (indices, packed
sequence construction in transposed layout) and the final scatter; the device
does all dense compute (GRU scans, LayerNorm-folded MLP head) in bf16 with
fp32 PSUM accumulation.

Device-side reformulation:
- Backward GRU runs t=T-1..0 over the SAME packed sequence (no rev()).
- h freezes at invalid steps via a +BIG bias on the z-gate (rank-1 matmul).
- out[t] = h after step t, unmasked; invalid packed positions are never
  read by the host scatter. hT_fwd = out_f[15], hT_bwd = out_b[0].
- LayerNorm folded into the W1 matmul: pre = rstd*(W1g@x - mu*u) + c0.
"""

import numpy as np
import ml_dtypes

import birfix

birfix.install()

import concourse.bass as bass
import concourse.mybir as mybir
import concourse.tile as tile
from concourse.bass_utils import run_bass_kernel_spmd

N_AGENTS = 8
ND = 16
B = 1024
P = 128
H = 256
P2 = 256        # 2*PLAN
G3 = 768        # 3*H
BH = 512        # half-batch columns processed per chunk
NHALF = B // BH
BIG = 40.0
F32 = mybir.dt.float32
BF16 = mybir.dt.bfloat16
AF = mybir.ActivationFunctionType

_cache = {}


def build_bass():
    nc = bass.Bass()
    # register the LN-eps constant for activation bias use
    _eps_t = nc.alloc_sbuf_tensor("const-float32-eps", [128, 1], F32)
    nc.gpsimd.memset(_eps_t.ap(), 1e-5)
    nc.const_aps.aps[(F32, 1e-5)] = _eps_t.ap()
    nc.all_engine_barrier()
    # inputs (per-core = per-agent); all partition-major layouts
    cseqT = nc.dram_tensor("cseqT", [P, 2, ND, B], BF16, kind="ExternalInput")
    wiT = nc.dram_tensor("wiT", [P, 2, 2, G3], BF16, kind="ExternalInput")
    whT = nc.dram_tensor("whT", [P, 2, 2, G3], BF16, kind="ExternalInput")
    h0T = nc.dram_tensor("h0T", [P, 2, 2, B], BF16, kind="ExternalInput")
    maskbig = nc.dram_tensor("maskbig", [1, ND, B], BF16, kind="ExternalInput")
    onesk1 = nc.dram_tensor("onesk1", [1, P], BF16, kind="ExternalInput")
    ones128 = nc.dram_tensor("ones128", [P, P], BF16, kind="ExternalInput")
    ident = nc.dram_tensor("ident", [P, P], BF16, kind="ExternalInput")
    w1gT = nc.dram_tensor("w1gT", [P, 4, H], BF16, kind="ExternalInput")
    unegT = nc.dram_tensor("unegT", [1, H], BF16, kind="ExternalInput")
    w2T = nc.dram_tensor("w2T", [P, 2, 2], BF16, kind="ExternalInput")
    # outputs
    cm = nc.dram_tensor("cm", [2, ND, B], F32, kind="ExternalOutput")
    stats = nc.dram_tensor("stats", [2, ND, B], F32, kind="ExternalOutput")
    hxs = nc.dram_tensor("hxs", [P, 2, 2, B], F32, kind="ExternalOutput")

    with tile.TileContext(nc) as tc:
        with (
            tc.tile_pool(name="wpool", bufs=1) as wpool,
            tc.tile_pool(name="outp", bufs=1) as outp,
            tc.tile_pool(name="xs", bufs=3) as xs,
            tc.tile_pool(name="ew", bufs=2) as ew,
            tc.tile_pool(name="hd", bufs=2) as hd,
            tc.tile_pool(name="ps", bufs=1, space="PSUM") as ps,
        ):
            # resident weights / constants
            wi = wpool.tile([P, 2, 2, G3], BF16, tag="wi")
            nc.sync.dma_start(wi[:], wiT[:])
            wh = wpool.tile([P, 2, 2, G3], BF16, tag="wh")
            nc.sync.dma_start(wh[:], whT[:])
            h0 = wpool.tile([P, 2, 2, B], BF16, tag="h0")
            nc.sync.dma_start(h0[:], h0T[:])
            mb = wpool.tile([1, ND, B], BF16, tag="mb")
            nc.sync.dma_start(mb[:], maskbig[:])
            o1 = wpool.tile([1, P], BF16, tag="o1")
            nc.sync.dma_start(o1[:], onesk1[:])
            o128 = wpool.tile([P, P], BF16, tag="o128")
            nc.sync.dma_start(o128[:], ones128[:])
            idn = wpool.tile([P, P], BF16, tag="idn")
            nc.sync.dma_start(idn[:], ident[:])
            w1g = wpool.tile([P, 4, H], BF16, tag="w1g")
            nc.sync.dma_start(w1g[:], w1gT[:])
            un = wpool.tile([1, H], BF16, tag="un")
            nc.sync.dma_start(un[:], unegT[:])
            w2 = wpool.tile([P, 2, 2], BF16, tag="w2")
            nc.sync.dma_start(w2[:], w2T[:])

            for hb in range(NHALF):
                col = hb * BH
                # out buffers per dir: (P, ND, 2kchunk, BH) bf16
                outs = [outp.tile([P, ND, 2, BH], BF16, tag=f"out{d}",
                                  name=f"out{d}") for d in range(2)]

                tord = [list(range(ND)), list(range(ND - 1, -1, -1))]
                for step in range(ND):
                    for d in range(2):
                        t = tord[d][step]
                        xt = [xs.tile([P, BH], BF16, tag=f"x{k}", name=f"x{k}")
                              for k in range(2)]
                        for k in range(2):
                            nc.sync.dma_start(xt[k][:], cseqT[:, k, t, col:col + BH])
                        if step == 0:
                            hprev = [h0[:, d, k, col:col + BH] for k in range(2)]
                            hprev_all = h0[:, d, :, col:col + BH]
                        else:
                            tp = tord[d][step - 1]
                            hprev = [outs[d][:, tp, k, :] for k in range(2)]
                            hprev_all = outs[d][:, tp, :, :]
                        # r,z gates: one 4-bank psum tile (m=0..3)
                        rz = ps.tile([P, 4, BH], F32, tag="rz4", name="rz4")
                        for m in range(4):
                            mm = m * P
                            ops = [(wi[:, d, 0, mm:mm + P], xt[0][:]),
                                   (wi[:, d, 1, mm:mm + P], xt[1][:]),
                                   (wh[:, d, 0, mm:mm + P], hprev[0]),
                                   (wh[:, d, 1, mm:mm + P], hprev[1])]
                            if m >= 2:
                                ops.append((o1[:, 0:P], mb[:, t, col:col + BH]))
                            for j, (lh, rh) in enumerate(ops):
                                nc.tensor.matmul(rz[:, m, :], lh, rh, start=(j == 0),
                                                 stop=(j == len(ops) - 1))
                        rt = ew.tile([P, 2, BH], BF16, tag="r", name="rt")
                        nc.scalar.activation(rt[:], rz[:, 0:2, :], AF.Sigmoid)
                        zt = ew.tile([P, 2, BH], BF16, tag="z", name="zt")
                        nc.scalar.activation(zt[:], rz[:, 2:4, :], AF.Sigmoid)
                        z1t = ew.tile([P, 2, BH], BF16, tag="zi", name="z1t")
                        nc.scalar.activation(z1t[:], rz[:, 2:4, :], AF.Sigmoid,
                                             scale=-1.0)
                        # n gate: gh pair then gi pair
                        png = ps.tile([P, 2, BH], F32, tag="png", name="png")
                        pni = ps.tile([P, 2, BH], F32, tag="pni", name="pni")
                        for m in range(2):
                            mm = 4 * P + m * P
                            nc.tensor.matmul(png[:, m, :], wh[:, d, 0, mm:mm + P],
                                             hprev[0], start=True, stop=False)
                            nc.tensor.matmul(png[:, m, :], wh[:, d, 1, mm:mm + P],
                                             hprev[1], start=False, stop=True)
                            nc.tensor.matmul(pni[:, m, :], wi[:, d, 0, mm:mm + P],
                                             xt[0][:], start=True, stop=False)
                            nc.tensor.matmul(pni[:, m, :], wi[:, d, 1, mm:mm + P],
                                             xt[1][:], start=False, stop=True)
                        tgh = ew.tile([P, 2, BH], BF16, tag="tg", name="tgh")
                        nc.scalar.activation(tgh[:], png[:], AF.Copy)
                        t1 = ew.tile([P, 2, BH], BF16, tag="t1", name="t1")
                        nc.vector.tensor_mul(t1[:], rt[:], tgh[:])
                        npre = ew.tile([P, 2, BH], BF16, tag="np", name="npre")
                        nc.vector.scalar_tensor_tensor(
                            npre[:], t1[:], 0.0, pni[:],
                            mybir.AluOpType.bypass, mybir.AluOpType.add)
                        nt = ew.tile([P, 2, BH], BF16, tag="n", name="nt")
                        nc.scalar.activation(nt[:], npre[:], AF.Tanh)
                        a1 = ew.tile([P, 2, BH], BF16, tag="a", name="a1")
                        nc.vector.tensor_mul(a1[:], zt[:], hprev_all)
                        a2 = ew.tile([P, 2, BH], BF16, tag="b", name="a2")
                        nc.vector.tensor_mul(a2[:], z1t[:], nt[:])
                        nc.vector.tensor_add(outs[d][:, t, :, :], a1[:], a2[:])
                for d in range(2):
                    tf = ND - 1 if d == 0 else 0
                    for k in range(2):
                        hf = hd.tile([P, BH], F32, tag="hx", name="hf")
                        nc.vector.tensor_copy(hf[:], outs[d][:, tf, k, :])
                        nc.sync.dma_start(hxs[:, d, k, col:col + BH], hf[:])

                # ---- head (folded LayerNorm + MLP) ----
                for t in range(ND):
                    S = [outs[0][:, t, 0, :], outs[0][:, t, 1, :],
                         outs[1][:, t, 0, :], outs[1][:, t, 1, :]]
                    pst = ps.tile([P, 2, BH], F32, tag="png", name="pst")
                    for j in range(4):
                        nc.tensor.matmul(pst[:, 0, :], o128[:], S[j], start=(j == 0),
                                         stop=(j == 3))
                    sqs = []
                    for d in range(2):
                        sq = hd.tile([P, 2, BH], BF16, tag="sq", name="sq")
                        nc.scalar.activation(sq[:], outs[d][:, t, :, :], AF.Square)
                        sqs.append(sq)
                    for j in range(4):
                        nc.tensor.matmul(pst[:, 1, :], o128[:], sqs[j // 2][:, j % 2, :],
                                         start=(j == 0), stop=(j == 3))
                    st0 = hd.tile([1, BH], F32, tag="st0", name="st0")
                    nc.scalar.activation(st0[:], pst[0:1, 0, :], AF.Copy)
                    nc.sync.dma_start(stats[0:1, t, col:col + BH], st0[:])
                    st1 = hd.tile([1, BH], F32, tag="st1", name="st1")
                    nc.scalar.activation(st1[:], pst[0:1, 1, :], AF.Copy)
                    nc.sync.dma_start(stats[1:2, t, col:col + BH], st1[:])
                    mur = hd.tile([1, BH], BF16, tag="mur", name="mur")
                    nc.scalar.activation(mur[:], pst[0:1, 0, :], AF.Copy)
                    pa = ps.tile([P, 2, BH], F32, tag="pni", name="pa")
                    for m in range(2):
                        for k in range(4):
                            nc.tensor.matmul(pa[:, m, :], w1g[:, k, m * P:m * P + P],
                                             S[k], start=(k == 0), stop=False)
                        nc.tensor.matmul(pa[:, m, :], un[:, m * P:m * P + P], mur[:],
                                         start=False, stop=True)
                    h1t = hd.tile([P, 2, BH], BF16, tag="h1", name="h1t")
                    nc.scalar.activation(h1t[:], pa[:], AF.Relu)
                    pcm = ps.tile([2, BH], F32, tag="rz4", name="pcm")
                    nc.tensor.matmul(pcm[:], w2[:, 0, 0:2], h1t[:, 0, :],
                                     start=True, stop=False)
                    nc.tensor.matmul(pcm[:], w2[:, 1, 0:2], h1t[:, 1, :],
                                     start=False, stop=True)
                    cmt = hd.tile([2, BH], F32, tag="cmt", name="cmt")
                    nc.scalar.activation(cmt[:], pcm[:], AF.Copy)
                    nc.sync.dma_start(cm[:, t, col:col + BH], cmt[:])
    return nc


def host_prep(inputs):
    plans = np.asarray(inputs["plans"], np.float32)
    comm_plans = np.asarray(inputs["comm_plans"], np.float32)
    empty = ~np.any(comm_plans != 0, axis=(1, 2))
    comm = np.where(empty[:, None, None], plans, comm_plans)
    mask_real = np.any(comm != 0, -1).T
    mask = np.concatenate([mask_real, np.ones((ND - N_AGENTS, B), bool)], 0)
    mask_flat = mask.reshape(-1)
    seq_len = mask.sum(0)
    packed_mask = np.arange(ND)[:, None] < seq_len[None, :]
    pm_flat = packed_mask.reshape(-1)
    src_idx = np.nonzero(mask_flat)[0]
    gather_idx = np.zeros(ND * B, dtype=np.int64)
    gather_idx[pm_flat] = src_idx
    return comm, mask_flat, packed_mask, pm_flat, gather_idx


def kernel(**inputs):
    comm, mask_flat, packed_mask, pm_flat, gather_idx = host_prep(inputs)
    plans = np.asarray(inputs["plans"], np.float32)
    dn = np.asarray(inputs["dummy_noise"], np.float32)
    ch = np.asarray(inputs["coord_hiddens"], np.float32)
    mbig = (BIG * (1.0 - packed_mask.astype(np.float32)))[None]  # (1,ND,B)
    bf = ml_dtypes.bfloat16

    in_maps = []
    for i in range(N_AGENTS):
        plan_i = plans[:, i]
        others = np.concatenate(
            [np.broadcast_to(plan_i[:, None, :], comm.shape), comm], -1)
        x = np.concatenate([others.transpose(1, 0, 2), dn[i]], 0)   # (ND,B,2P)
        cseq = x.reshape(ND * B, P2)[gather_idx]                     # (ND*B,2P)
        # (ND,B,2P) -> (2P, ND, B) -> (2,P,ND,B) -> (P,2,ND,B)
        cseqT = np.ascontiguousarray(
            cseq.reshape(ND, B, P2).transpose(2, 0, 1)
        ).reshape(2, P, ND, B).transpose(1, 0, 2, 3)

        Wi_f = np.asarray(inputs["Wi_f"][i], np.float32)
        Wh_f = np.asarray(inputs["Wh_f"][i], np.float32)
        Wi_b = np.asarray(inputs["Wi_b"][i], np.float32)
        Wh_b = np.asarray(inputs["Wh_b"][i], np.float32)
        g = np.asarray(inputs["ln_g"][i], np.float32)
        beta = np.asarray(inputs["ln_b"][i], np.float32)
        W1 = np.asarray(inputs["W1"][i], np.float32)
        b1 = np.asarray(inputs["b1"][i], np.float32)
        W2 = np.asarray(inputs["W2"][i], np.float32)

        # (d,k,P,G3) -> (P,d,k,G3)
        wiT = np.stack([Wi_f.T.reshape(2, P, G3),
                        Wi_b.T.reshape(2, P, G3)]).transpose(2, 0, 1, 3)
        whT = np.stack([Wh_f.T.reshape(2, P, G3),
                        Wh_b.T.reshape(2, P, G3)]).transpose(2, 0, 1, 3)
        # ch[i]: (2,B,H) -> (d,H,B) -> (d,2,P,B) -> (P,d,2,B)
        h0T = np.ascontiguousarray(ch[i].transpose(0, 2, 1)).reshape(
            2, 2, P, B).transpose(2, 0, 1, 3)
        W1g = W1 * g[None, :]                    # (H, 2H)
        w1gT = np.ascontiguousarray(W1g.T).reshape(4, P, H).transpose(1, 0, 2)
        u = W1g.sum(1)                           # (H,)
        unegT = (-u / 512.0)[None, :]            # (1, H)
        assert np.allclose(W1 @ beta + b1, 0.0), "c0 fold requires zero bias"
        w2T = np.ascontiguousarray(W2.T).reshape(2, P, 2).transpose(1, 0, 2)

        in_maps.append(dict(
            cseqT=np.ascontiguousarray(cseqT).astype(bf),
            wiT=np.ascontiguousarray(wiT).astype(bf),
            whT=np.ascontiguousarray(whT).astype(bf),
            h0T=np.ascontiguousarray(h0T).astype(bf),
            maskbig=mbig.astype(bf),
            onesk1=np.ones((1, P), bf),
            ones128=np.ones((P, P), bf),
            ident=np.eye(P, dtype=np.float32).astype(bf),
            w1gT=np.ascontiguousarray(w1gT).astype(bf),
            unegT=unegT.astype(bf),
            w2T=np.ascontiguousarray(w2T).astype(bf),
        ))

    if "nc" not in _cache:
        _cache["nc"] = build_bass()
    nc = _cache["nc"]
    import os
    trace = bool(os.environ.get("KTRACE"))
    if trace:
        import hookfix
        hookfix.install()
    res = run_bass_kernel_spmd(nc, in_maps, core_ids=list(range(N_AGENTS)),
                               trace=trace)
    _cache["last_results"] = res

    coord_masks = np.zeros((N_AGENTS, ND, B, 2), np.float32)
    coord_hxs = np.zeros((N_AGENTS, 2, B, H), np.float32)
    for i in range(N_AGENTS):
        out = res.results[i]
        cmT = out["cm"].reshape(2, ND * B)       # (2, ND*B)
        hxsT = out["hxs"]                        # (P,2,2,B)
        W1 = np.asarray(inputs["W1"][i], np.float32)
        b1 = np.asarray(inputs["b1"][i], np.float32)
        W2 = np.asarray(inputs["W2"][i], np.float32)
        b2 = np.asarray(inputs["b2"][i], np.float32)
        beta = np.asarray(inputs["ln_b"][i], np.float32)
        cm_inv = W2 @ np.maximum(W1 @ beta + b1, 0.0) + b2
        resf = np.broadcast_to(cm_inv, (ND * B, 2)).copy()
        stT = out["stats"].reshape(2, ND * B)
        mu = stT[0] / 512.0
        var = stT[1] / 512.0 - mu * mu
        rstd = 1.0 / np.sqrt(var + 1e-5)          # (ND*B,)
        cmvals = cmT.T * rstd[:, None] + b2       # (ND*B, 2) head per packed slot
        cmvals[~pm_flat] = cm_inv                 # ref zeroes invalid packed slots
        K = int(mask_flat.sum())
        resf[mask_flat] = cmvals[:K]
        coord_masks[i] = resf.reshape(ND, B, 2)
        # hxsT (P,d,k,B) -> (d, B, k*P+p)
        coord_hxs[i] = hxsT.transpose(1, 3, 2, 0).reshape(2, B, H)
    return coord_masks, coord_hxs


# revision 14
# speedup vs baseline: 1.3024x; 1.3024x over previous
"""Trainium2 Bass kernel for nn_Coordinator (ragged bidirectional GRU coordinator).

Sharding: agent-parallel — 8 agents over 8 NeuronCores (one agent per core),
full batch per core. Host does the ragged pack bookkeeping (indices, packed
sequence construction in transposed layout) and the final scatter; the device
does all dense compute (GRU scans, LayerNorm-folded MLP head) in bf16 with
fp32 PSUM accumulation.

Device-side reformulation:
- Backward GRU runs t=T-1..0 over the SAME packed sequence (no rev()).
- h freezes at invalid steps via a +BIG bias on the z-gate (rank-1 matmul).
- out[t] = h after step t, unmasked; invalid packed positions are never
  read by the host scatter. hT_fwd = out_f[15], hT_bwd = out_b[0].
- LayerNorm folded into the W1 matmul: pre = rstd*(W1g@x - mu*u) + c0.
"""

import numpy as np
import ml_dtypes

import birfix

birfix.install()

import concourse.bass as bass
import concourse.mybir as mybir
import concourse.tile as tile
from concourse.bass_utils import run_bass_kernel_spmd

N_AGENTS = 8
ND = 16
B = 1024
P = 128
H = 256
P2 = 256        # 2*PLAN
G3 = 768        # 3*H
BH = 512        # half-batch columns processed per chunk
NHALF = B // BH
BIG = 40.0
F32 = mybir.dt.float32
BF16 = mybir.dt.bfloat16
AF = mybir.ActivationFunctionType

_cache = {}


def build_bass():
    nc = bass.Bass()
    # register the LN-eps constant for activation bias use
    _eps_t = nc.alloc_sbuf_tensor("const-float32-eps", [128, 1], F32)
    nc.gpsimd.memset(_eps_t.ap(), 1e-5)
    nc.const_aps.aps[(F32, 1e-5)] = _eps_t.ap()
    nc.all_engine_barrier()
    # inputs (per-core = per-agent); all partition-major layouts
    cseqT = nc.dram_tensor("cseqT", [P, 2, ND, B], BF16, kind="ExternalInput")
    wiT = nc.dram_tensor("wiT", [P, 2, 2, G3], BF16, kind="ExternalInput")
    whT = nc.dram_tensor("whT", [P, 2, 2, G3], BF16, kind="ExternalInput")
    h0T = nc.dram_tensor("h0T", [P, 2, 2, B], BF16, kind="ExternalInput")
    maskbig = nc.dram_tensor("maskbig", [1, ND, B], BF16, kind="ExternalInput")
    onesk1 = nc.dram_tensor("onesk1", [1, P], BF16, kind="ExternalInput")
    ones128 = nc.dram_tensor("ones128", [P, P], BF16, kind="ExternalInput")
    ident = nc.dram_tensor("ident", [P, P], BF16, kind="ExternalInput")
    w1gT = nc.dram_tensor("w1gT", [P, 4, H], BF16, kind="ExternalInput")
    unegT = nc.dram_tensor("unegT", [1, H], BF16, kind="ExternalInput")
    w2T = nc.dram_tensor("w2T", [P, 2, 2], BF16, kind="ExternalInput")
    # outputs
    cm = nc.dram_tensor("cm", [2, ND, B], F32, kind="ExternalOutput")
    stats = nc.dram_tensor("stats", [2, ND, B], F32, kind="ExternalOutput")
    hxs = nc.dram_tensor("hxs", [P, 2, 2, B], F32, kind="ExternalOutput")

    with tile.TileContext(nc) as tc:
        with (
            tc.tile_pool(name="wpool", bufs=1) as wpool,
            tc.tile_pool(name="outp", bufs=1) as outp,
            tc.tile_pool(name="xs", bufs=3) as xs,
            tc.tile_pool(name="ew", bufs=3) as ew,
            tc.tile_pool(name="hd", bufs=2) as hd,
            tc.tile_pool(name="ps", bufs=4, space="PSUM") as ps,
        ):
            # resident weights / constants
            wi = wpool.tile([P, 2, 2, G3], BF16, tag="wi")
            nc.sync.dma_start(wi[:], wiT[:])
            wh = wpool.tile([P, 2, 2, G3], BF16, tag="wh")
            nc.sync.dma_start(wh[:], whT[:])
            h0 = wpool.tile([P, 2, 2, B], BF16, tag="h0")
            nc.sync.dma_start(h0[:], h0T[:])
            mb = wpool.tile([1, ND, B], BF16, tag="mb")
            nc.sync.dma_start(mb[:], maskbig[:])
            o1 = wpool.tile([1, P], BF16, tag="o1")
            nc.sync.dma_start(o1[:], onesk1[:])
            o128 = wpool.tile([P, P], BF16, tag="o128")
            nc.sync.dma_start(o128[:], ones128[:])
            idn = wpool.tile([P, P], BF16, tag="idn")
            nc.sync.dma_start(idn[:], ident[:])
            w1g = wpool.tile([P, 4, H], BF16, tag="w1g")
            nc.sync.dma_start(w1g[:], w1gT[:])
            un = wpool.tile([1, H], BF16, tag="un")
            nc.sync.dma_start(un[:], unegT[:])
            w2 = wpool.tile([P, 2, 2], BF16, tag="w2")
            nc.sync.dma_start(w2[:], w2T[:])

            for hb in range(NHALF):
                col = hb * BH
                # out buffers per dir: (P, ND, 2kchunk, BH) bf16
                outs = [outp.tile([P, ND, 2, BH], BF16, tag=f"out{d}",
                                  name=f"out{d}") for d in range(2)]

                tord = [list(range(ND)), list(range(ND - 1, -1, -1))]
                for step in range(ND):
                    for d in range(2):
                        t = tord[d][step]
                        xt = [xs.tile([P, BH], BF16, tag=f"x{k}", name=f"x{k}")
                              for k in range(2)]
                        for k in range(2):
                            nc.sync.dma_start(xt[k][:], cseqT[:, k, t, col:col + BH])
                        if step == 0:
                            hprev = [h0[:, d, k, col:col + BH] for k in range(2)]
                            hprev_all = h0[:, d, :, col:col + BH]
                        else:
                            tp = tord[d][step - 1]
                            hprev = [outs[d][:, tp, k, :] for k in range(2)]
                            hprev_all = outs[d][:, tp, :, :]
                        # r,z gates: one 4-bank psum tile (m=0..3)
                        rza = ps.tile([P, 2, BH], F32, tag="g2", name="rza")
                        rzb = ps.tile([P, 2, BH], F32, tag="g2", name="rzb")
                        for m in range(4):
                            mm = m * P
                            dst = rza[:, m, :] if m < 2 else rzb[:, m - 2, :]
                            ops = [(wi[:, d, 0, mm:mm + P], xt[0][:]),
                                   (wi[:, d, 1, mm:mm + P], xt[1][:]),
                                   (wh[:, d, 0, mm:mm + P], hprev[0]),
                                   (wh[:, d, 1, mm:mm + P], hprev[1])]
                            if m >= 2:
                                ops.append((o1[:, 0:P], mb[:, t, col:col + BH]))
                            for j, (lh, rh) in enumerate(ops):
                                nc.tensor.matmul(dst, lh, rh, start=(j == 0),
                                                 stop=(j == len(ops) - 1))
                        rt = ew.tile([P, 2, BH], BF16, tag="r", name="rt")
                        nc.scalar.activation(rt[:], rza[:], AF.Sigmoid)
                        zt = ew.tile([P, 2, BH], BF16, tag="z", name="zt")
                        nc.scalar.activation(zt[:], rzb[:], AF.Sigmoid)
                        z1t = ew.tile([P, 2, BH], BF16, tag="zi", name="z1t")
                        nc.scalar.activation(z1t[:], rzb[:], AF.Sigmoid,
                                             scale=-1.0)
                        # n gate: gh pair then gi pair
                        png = ps.tile([P, 2, BH], F32, tag="g2", name="png")
                        pni = ps.tile([P, 2, BH], F32, tag="g2", name="pni")
                        for m in range(2):
                            mm = 4 * P + m * P
                            nc.tensor.matmul(png[:, m, :], wh[:, d, 0, mm:mm + P],
                                             hprev[0], start=True, stop=False)
                            nc.tensor.matmul(png[:, m, :], wh[:, d, 1, mm:mm + P],
                                             hprev[1], start=False, stop=True)
                            nc.tensor.matmul(pni[:, m, :], wi[:, d, 0, mm:mm + P],
                                             xt[0][:], start=True, stop=False)
                            nc.tensor.matmul(pni[:, m, :], wi[:, d, 1, mm:mm + P],
                                             xt[1][:], start=False, stop=True)
                        tgh = ew.tile([P, 2, BH], BF16, tag="tg", name="tgh")
                        nc.scalar.activation(tgh[:], png[:], AF.Copy)
                        t1 = ew.tile([P, 2, BH], BF16, tag="t1", name="t1")
                        nc.vector.tensor_mul(t1[:], rt[:], tgh[:])
                        npre = ew.tile([P, 2, BH], BF16, tag="np", name="npre")
                        nc.vector.scalar_tensor_tensor(
                            npre[:], t1[:], 0.0, pni[:],
                            mybir.AluOpType.bypass, mybir.AluOpType.add)
                        nt = ew.tile([P, 2, BH], BF16, tag="n", name="nt")
                        nc.scalar.activation(nt[:], npre[:], AF.Tanh)
                        a1 = ew.tile([P, 2, BH], BF16, tag="a", name="a1")
                        nc.vector.tensor_mul(a1[:], zt[:], hprev_all)
                        a2 = ew.tile([P, 2, BH], BF16, tag="b", name="a2")
                        nc.vector.tensor_mul(a2[:], z1t[:], nt[:])
                        nc.vector.tensor_add(outs[d][:, t, :, :], a1[:], a2[:])
                for d in range(2):
                    tf = ND - 1 if d == 0 else 0
                    for k in range(2):
                        hf = hd.tile([P, BH], F32, tag="hx", name="hf")
                        nc.vector.tensor_copy(hf[:], outs[d][:, tf, k, :])
                        nc.sync.dma_start(hxs[:, d, k, col:col + BH], hf[:])

                # ---- head (folded LayerNorm + MLP) ----
                for t in range(ND):
                    S = [outs[0][:, t, 0, :], outs[0][:, t, 1, :],
                         outs[1][:, t, 0, :], outs[1][:, t, 1, :]]
                    pst = ps.tile([P, 2, BH], F32, tag="g2", name="pst")
                    for j in range(4):
                        nc.tensor.matmul(pst[:, 0, :], o128[:], S[j], start=(j == 0),
                                         stop=(j == 3))
                    sqs = []
                    for d in range(2):
                        sq = hd.tile([P, 2, BH], BF16, tag="sq", name="sq")
                        nc.scalar.activation(sq[:], outs[d][:, t, :, :], AF.Square)
                        sqs.append(sq)
                    for j in range(4):
                        nc.tensor.matmul(pst[:, 1, :], o128[:], sqs[j // 2][:, j % 2, :],
                                         start=(j == 0), stop=(j == 3))
                    st0 = hd.tile([1, BH], F32, tag="st0", name="st0")
                    nc.scalar.activation(st0[:], pst[0:1, 0, :], AF.Copy)
                    nc.sync.dma_start(stats[0:1, t, col:col + BH], st0[:])
                    st1 = hd.tile([1, BH], F32, tag="st1", name="st1")
                    nc.scalar.activation(st1[:], pst[0:1, 1, :], AF.Copy)
                    nc.sync.dma_start(stats[1:2, t, col:col + BH], st1[:])
                    mur = hd.tile([1, BH], BF16, tag="mur", name="mur")
                    nc.scalar.activation(mur[:], pst[0:1, 0, :], AF.Copy)
                    pa = ps.tile([P, 2, BH], F32, tag="g2", name="pa")
                    for m in range(2):
                        for k in range(4):
                            nc.tensor.matmul(pa[:, m, :], w1g[:, k, m * P:m * P + P],
                                             S[k], start=(k == 0), stop=False)
                        nc.tensor.matmul(pa[:, m, :], un[:, m * P:m * P + P], mur[:],
                                         start=False, stop=True)
                    h1t = hd.tile([P, 2, BH], BF16, tag="h1", name="h1t")
                    nc.scalar.activation(h1t[:], pa[:], AF.Relu)
                    pcm = ps.tile([2, BH], F32, tag="g2", name="pcm")
                    nc.tensor.matmul(pcm[:], w2[:, 0, 0:2], h1t[:, 0, :],
                                     start=True, stop=False)
                    nc.tensor.matmul(pcm[:], w2[:, 1, 0:2], h1t[:, 1, :],
                                     start=False, stop=True)
                    cmt = hd.tile([2, BH], F32, tag="cmt", name="cmt")
                    nc.scalar.activation(cmt[:], pcm[:], AF.Copy)
                    nc.sync.dma_start(cm[:, t, col:col + BH], cmt[:])
    return nc


def host_prep(inputs):
    plans = np.asarray(inputs["plans"], np.float32)
    comm_plans = np.asarray(inputs["comm_plans"], np.float32)
    empty = ~np.any(comm_plans != 0, axis=(1, 2))
    comm = np.where(empty[:, None, None], plans, comm_plans)
    mask_real = np.any(comm != 0, -1).T
    mask = np.concatenate([mask_real, np.ones((ND - N_AGENTS, B), bool)], 0)
    mask_flat = mask.reshape(-1)
    seq_len = mask.sum(0)
    packed_mask = np.arange(ND)[:, None] < seq_len[None, :]
    pm_flat = packed_mask.reshape(-1)
    src_idx = np.nonzero(mask_flat)[0]
    gather_idx = np.zeros(ND * B, dtype=np.int64)
    gather_idx[pm_flat] = src_idx
    return comm, mask_flat, packed_mask, pm_flat, gather_idx


def kernel(**inputs):
    comm, mask_flat, packed_mask, pm_flat, gather_idx = host_prep(inputs)
    plans = np.asarray(inputs["plans"], np.float32)
    dn = np.asarray(inputs["dummy_noise"], np.float32)
    ch = np.asarray(inputs["coord_hiddens"], np.float32)
    mbig = (BIG * (1.0 - packed_mask.astype(np.float32)))[None]  # (1,ND,B)
    bf = ml_dtypes.bfloat16

    in_maps = []
    for i in range(N_AGENTS):
        plan_i = plans[:, i]
        others = np.concatenate(
            [np.broadcast_to(plan_i[:, None, :], comm.shape), comm], -1)
        x = np.concatenate([others.transpose(1, 0, 2), dn[i]], 0)   # (ND,B,2P)
        cseq = x.reshape(ND * B, P2)[gather_idx]                     # (ND*B,2P)
        # (ND,B,2P) -> (2P, ND, B) -> (2,P,ND,B) -> (P,2,ND,B)
        cseqT = np.ascontiguousarray(
            cseq.reshape(ND, B, P2).transpose(2, 0, 1)
        ).reshape(2, P, ND, B).transpose(1, 0, 2, 3)

        Wi_f = np.asarray(inputs["Wi_f"][i], np.float32)
        Wh_f = np.asarray(inputs["Wh_f"][i], np.float32)
        Wi_b = np.asarray(inputs["Wi_b"][i], np.float32)
        Wh_b = np.asarray(inputs["Wh_b"][i], np.float32)
        g = np.asarray(inputs["ln_g"][i], np.float32)
        beta = np.asarray(inputs["ln_b"][i], np.float32)
        W1 = np.asarray(inputs["W1"][i], np.float32)
        b1 = np.asarray(inputs["b1"][i], np.float32)
        W2 = np.asarray(inputs["W2"][i], np.float32)

        # (d,k,P,G3) -> (P,d,k,G3)
        wiT = np.stack([Wi_f.T.reshape(2, P, G3),
                        Wi_b.T.reshape(2, P, G3)]).transpose(2, 0, 1, 3)
        whT = np.stack([Wh_f.T.reshape(2, P, G3),
                        Wh_b.T.reshape(2, P, G3)]).transpose(2, 0, 1, 3)
        # ch[i]: (2,B,H) -> (d,H,B) -> (d,2,P,B) -> (P,d,2,B)
        h0T = np.ascontiguousarray(ch[i].transpose(0, 2, 1)).reshape(
            2, 2, P, B).transpose(2, 0, 1, 3)
        W1g = W1 * g[None, :]                    # (H, 2H)
        w1gT = np.ascontiguousarray(W1g.T).reshape(4, P, H).transpose(1, 0, 2)
        u = W1g.sum(1)                           # (H,)
        unegT = (-u / 512.0)[None, :]            # (1, H)
        assert np.allclose(W1 @ beta + b1, 0.0), "c0 fold requires zero bias"
        w2T = np.ascontiguousarray(W2.T).reshape(2, P, 2).transpose(1, 0, 2)

        in_maps.append(dict(
            cseqT=np.ascontiguousarray(cseqT).astype(bf),
            wiT=np.ascontiguousarray(wiT).astype(bf),
            whT=np.ascontiguousarray(whT).astype(bf),
            h0T=np.ascontiguousarray(h0T).astype(bf),
            maskbig=mbig.astype(bf),
            onesk1=np.ones((1, P), bf),
            ones128=np.ones((P, P), bf),
            ident=np.eye(P, dtype=np.float32).astype(bf),
            w1gT=np.ascontiguousarray(w1gT).astype(bf),
            unegT=unegT.astype(bf),
            w2T=np.ascontiguousarray(w2T).astype(bf),
        ))

    if "nc" not in _cache:
        _cache["nc"] = build_bass()
    nc = _cache["nc"]
    import os
    trace = bool(os.environ.get("KTRACE"))
    if trace:
        import hookfix
        hookfix.install()
    res = run_bass_kernel_spmd(nc, in_maps, core_ids=list(range(N_AGENTS)),
                               trace=trace)
    _cache["last_results"] = res

    coord_masks = np.zeros((N_AGENTS, ND, B, 2), np.float32)
    coord_hxs = np.zeros((N_AGENTS, 2, B, H), np.float32)
    for i in range(N_AGENTS):
        out = res.results[i]
        cmT = out["cm"].reshape(2, ND * B)       # (2, ND*B)
        hxsT = out["hxs"]                        # (P,2,2,B)
        W1 = np.asarray(inputs["W1"][i], np.float32)
        b1 = np.asarray(inputs["b1"][i], np.float32)
        W2 = np.asarray(inputs["W2"][i], np.float32)
        b2 = np.asarray(inputs["b2"][i], np.float32)
        beta = np.asarray(inputs["ln_b"][i], np.float32)
        cm_inv = W2 @ np.maximum(W1 @ beta + b1, 0.0) + b2
        resf = np.broadcast_to(cm_inv, (ND * B, 2)).copy()
        stT = out["stats"].reshape(2, ND * B)
        mu = stT[0] / 512.0
        var = stT[1] / 512.0 - mu * mu
        rstd = 1.0 / np.sqrt(var + 1e-5)          # (ND*B,)
        cmvals = cmT.T * rstd[:, None] + b2       # (ND*B, 2) head per packed slot
        cmvals[~pm_flat] = cm_inv                 # ref zeroes invalid packed slots
        K = int(mask_flat.sum())
        resf[mask_flat] = cmvals[:K]
        coord_masks[i] = resf.reshape(ND, B, 2)
        # hxsT (P,d,k,B) -> (d, B, k*P+p)
        coord_hxs[i] = hxsT.transpose(1, 3, 2, 0).reshape(2, B, H)
    return coord_masks, coord_hxs
